# revision 1
# baseline (speedup 1.0000x reference)
"""Axial attention block (B=8, C=512, H=W=128, 8 heads) on 8 Trainium2 cores.

Sharding: data-parallel over batch — one batch element per NeuronCore. Each
core runs both axial passes (width attention, then height attention) on its
(C, H, W) slice and produces the full residual sum xs + oh + ow.

Layout strategy per core:
  - Matmul inputs are pre-cast to bf16 on the host: xbf (C,H,W) for the width
    pass and xtbf (C,W,H) (h<->w swapped) for the height pass, so both passes
    read contiguous (c, seq, s) tiles.  Weights are passed pre-transposed
    (c_in, c_out) in bf16.
  - Width pass: sequences along w (one per h); computes acc1 = xs + ow into a
    DRAM scratch in natural (C,H,W) layout (contiguous writes).
  - Height pass: sequences along h (one per w); reads acc1 in (c, h, w-block)
    blocks, adds oh, writes the final output (block-strided DMA, 32B runs).

Per-sequence attention (S=128, dh=64): scoresT = K^T.T @ Q^T per head in
(s_k, s_q) layout; exp on ScalarE (softmax max-subtraction skipped — scores
are O(6) bounded); denominators via ones-matmul on TensorE (replicated rows);
reciprocal_approx_fast + normalize on VectorE/GpSimd; AV directly in
(c, s_q) layout; O-projection batched over 4 sequences.
"""
import numpy as np
import ml_dtypes

P = 128          # partitions
C = 512          # channels
S = 128          # sequence length (H and W)
NCB = C // P     # channel blocks
NH = 8           # heads
DH = C // NH     # head dim
HC = 8           # sequences per chunk
G = 4            # sequences per projection group
NCORES = 8

_BF16 = ml_dtypes.bfloat16

_PROG = None  # cached compiled Bass program


def _build_program():
    from contextlib import ExitStack
    import concourse.tile as tile
    from concourse import bacc, mybir

    f32 = mybir.dt.float32
    bf = mybir.dt.bfloat16
    Exp = mybir.ActivationFunctionType.Exp

    nc = bacc.Bacc("TRN2", target_bir_lowering=False, debug=False)

    xf = nc.dram_tensor("xf", [C, S, S], f32, kind="ExternalInput").ap()
    xbf = nc.dram_tensor("xbf", [C, S, S], bf, kind="ExternalInput").ap()
    xtbf = nc.dram_tensor("xtbf", [C, S, S], bf, kind="ExternalInput").ap()
    wnames = ["wq_w", "wk_w", "wv_w", "wo_w", "wq_h", "wk_h", "wv_h", "wo_h"]
    wt = {n: nc.dram_tensor(n, [C, C], bf, kind="ExternalInput").ap() for n in wnames}
    acc1 = nc.dram_tensor("acc1", [C, S, S], f32).ap()
    out = nc.dram_tensor("out", [C, S, S], f32, kind="ExternalOutput").ap()

    with tile.TileContext(nc) as tc, ExitStack() as ctx:
        const = ctx.enter_context(tc.tile_pool(name="const", bufs=1))
        src_pool = ctx.enter_context(tc.tile_pool(name="src", bufs=2))
        resid_pool = ctx.enter_context(tc.tile_pool(name="resid", bufs=2))
        stage_pool = ctx.enter_context(tc.tile_pool(name="stage", bufs=2))
        qk_pool = ctx.enter_context(tc.tile_pool(name="qk", bufs=2))
        vt_pool = ctx.enter_context(tc.tile_pool(name="vt", bufs=2))
        ot_pool = ctx.enter_context(tc.tile_pool(name="ot", bufs=2))
        et_pool = ctx.enter_context(tc.tile_pool(name="et", bufs=2))
        rr_pool = ctx.enter_context(tc.tile_pool(name="rr", bufs=2))
        proj_ps = ctx.enter_context(tc.tile_pool(name="proj_ps", bufs=2, space="PSUM"))
        attn_ps = ctx.enter_context(tc.tile_pool(name="attn_ps", bufs=5, space="PSUM"))

        # resident weights: per matrix, NCB tiles of (128, C) = W.T[ci]
        w_sb = {}
        for n in wnames:
            tiles = []
            for ci in range(NCB):
                t = const.tile([P, C], bf, tag=f"w_{n}_{ci}", name=f"w_{n}_{ci}")
                nc.sync.dma_start(out=t, in_=wt[n][ci * P:(ci + 1) * P, :])
                tiles.append(t)
            w_sb[n] = tiles
        ones_sb = const.tile([P, P], bf, tag="ones", name="ones")
        nc.vector.memset(ones_sb, 1.0)

        def axial_pass(src_bf, wq, wk, wv, wo, resid, dest, transposed):
            """One axial attention pass over 128 sequences.

            src_bf: DRAM bf16 (C, NSEQ, S) — dim1 = sequence index, dim2 = pos.
            resid/dest: DRAM f32 in natural (C, H, W) layout.
            transposed=False: seq = h (width attention); blocks are (c, seq, s).
            transposed=True:  seq = w (height attention); resid/dest blocks are
                              (c, s, seq) in memory.
            """
            for chunk in range(S // HC):
                q0 = chunk * HC
                src_t = []
                for cb in range(NCB):
                    t = src_pool.tile([P, HC, S], bf, tag=f"src{cb}", name=f"src{cb}")
                    nc.sync.dma_start(
                        out=t, in_=src_bf[cb * P:(cb + 1) * P, q0:q0 + HC, :])
                    src_t.append(t)
                resid_t = []
                stage_t = []
                for cb in range(NCB):
                    cs = slice(cb * P, (cb + 1) * P)
                    if not transposed:
                        rt = resid_pool.tile([P, HC, S], f32, tag=f"res{cb}", name=f"res{cb}")
                        nc.sync.dma_start(out=rt, in_=resid[cs, q0:q0 + HC, :])
                        st = stage_pool.tile([P, HC, S], f32, tag=f"stg{cb}", name=f"stg{cb}")
                    else:
                        rt = resid_pool.tile([P, S, HC], f32, tag=f"res{cb}", name=f"res{cb}")
                        nc.sync.dma_start(out=rt, in_=resid[cs, :, q0:q0 + HC])
                        st = stage_pool.tile([P, S, HC], f32, tag=f"stg{cb}", name=f"stg{cb}")
                    resid_t.append(rt)
                    stage_t.append(st)

                for g in range(HC // G):
                    s0 = g * G
                    gsl = slice(s0, s0 + G)

                    # Q^T, K^T projections batched over the group: (c_out, G*S)
                    qt_sb, kt_sb = [], []
                    for wmat, dst_list, nm in ((wq, qt_sb, "qt"), (wk, kt_sb, "kt")):
                        for co in range(NCB):
                            pp = proj_ps.tile([P, G * S], f32, tag="proj", name="pp")
                            for ci in range(NCB):
                                nc.tensor.matmul(
                                    pp,
                                    lhsT=wmat[ci][:, co * P:(co + 1) * P],
                                    rhs=src_t[ci][:, gsl, :],
                                    start=(ci == 0), stop=(ci == NCB - 1))
                            sb_t = locals_pool_tile = qk_pool.tile(
                                [P, G * S], bf, tag=f"{nm}{co}", name=f"{nm}{co}")
                            nc.scalar.copy(sb_t, pp)
                            dst_list.append(sb_t)

                    # V per sequence: (s, c) layout
                    vt_sb = []
                    for sq in range(G):
                        pv = proj_ps.tile([P, C], f32, tag="proj", name="pv")
                        for ci in range(NCB):
                            nc.tensor.matmul(
                                pv, lhsT=src_t[ci][:, s0 + sq, :], rhs=wv[ci],
                                start=(ci == 0), stop=(ci == NCB - 1))
                        vt = vt_pool.tile([P, C], bf, tag=f"vt{sq}", name=f"vt{sq}")
                        nc.vector.tensor_copy(vt, pv)
                        vt_sb.append(vt)

                    # attention per sequence
                    ot_full = ot_pool.tile([P, NCB, G * S], bf, tag="ot", name="ot")
                    for sq in range(G):
                        ssl = slice(sq * S, (sq + 1) * S)
                        # scoresT: head h -> psum tile (h%2), col block h//2
                        stA = attn_ps.tile([P, 512], f32, tag="attn", name="stA")
                        stB = attn_ps.tile([P, 512], f32, tag="attn", name="stB")
                        for h in range(NH):
                            par, cb = h % 2, h // 2
                            rows = slice(par * DH, (par + 1) * DH)
                            dst = stA if par == 0 else stB
                            nc.tensor.matmul(
                                dst[:, cb * S:(cb + 1) * S],
                                lhsT=kt_sb[h // 2][rows, ssl],
                                rhs=qt_sb[h // 2][rows, ssl],
                                start=True, stop=True)
                        # exp (scale folds in dh^-0.5); max-subtraction skipped
                        et = et_pool.tile([P, 2, 512], bf, tag="et", name="et")
                        nc.scalar.activation(out=et[:, 0, :], in_=stA, func=Exp, scale=DH ** -0.5)
                        nc.scalar.activation(out=et[:, 1, :], in_=stB, func=Exp, scale=DH ** -0.5)
                        # denominators: ones-matmul -> replicated row sums
                        rA = attn_ps.tile([P, 512], f32, tag="attn", name="rA")
                        rB = attn_ps.tile([P, 512], f32, tag="attn", name="rB")
                        nc.tensor.matmul(rA, lhsT=ones_sb, rhs=et[:, 0, :], start=True, stop=True)
                        nc.tensor.matmul(rB, lhsT=ones_sb, rhs=et[:, 1, :], start=True, stop=True)
                        rrA = rr_pool.tile([P, 512], f32, tag="rrA", name="rrA")
                        rrB = rr_pool.tile([P, 512], f32, tag="rrB", name="rrB")
                        nc.vector.reciprocal_approx_fast(out=rrA, in_=rA)
                        nc.vector.reciprocal_approx_fast(out=rrB, in_=rB)
                        etn = et_pool.tile([P, 2, 512], bf, tag="etn", name="etn")
                        nc.vector.tensor_mul(etn[:, 0, :], et[:, 0, :], rrA)
                        nc.vector.tensor_mul(etn[:, 1, :], et[:, 1, :], rrB)
                        # AV: O'^T (c, s_q) packed in one bank
                        po = attn_ps.tile([P, 512], f32, tag="attn", name="po")
                        for h in range(NH):
                            par, cb = h % 2, h // 2
                            nc.tensor.matmul(
                                po[par * DH:(par + 1) * DH, cb * S:(cb + 1) * S],
                                lhsT=vt_sb[sq][:, h * DH:(h + 1) * DH],
                                rhs=etn[:, par, cb * S:(cb + 1) * S],
                                start=True, stop=True)
                        nc.scalar.copy(
                            ot_full[:, :, ssl],
                            po.rearrange("p (c s) -> p c s", c=NCB))

                    # O-projection batched over the group + residual + stage
                    for co in range(NCB):
                        pod = proj_ps.tile([P, G * S], f32, tag="proj", name="pod")
                        for ci in range(NCB):
                            nc.tensor.matmul(
                                pod,
                                lhsT=wo[ci][:, co * P:(co + 1) * P],
                                rhs=ot_full[:, ci, :],
                                start=(ci == 0), stop=(ci == NCB - 1))
                        pod3 = pod.rearrange("p (q s) -> p q s", q=G)
                        if not transposed:
                            nc.vector.tensor_add(
                                stage_t[co][:, gsl, :], pod3, resid_t[co][:, gsl, :])
                        else:
                            nc.vector.tensor_add(
                                stage_t[co][:, :, gsl].rearrange("p s q -> p q s"),
                                pod3,
                                resid_t[co][:, :, gsl].rearrange("p s q -> p q s"))

                for cb in range(NCB):
                    cs = slice(cb * P, (cb + 1) * P)
                    if not transposed:
                        nc.sync.dma_start(out=dest[cs, q0:q0 + HC, :], in_=stage_t[cb])
                    else:
                        nc.sync.dma_start(out=dest[cs, :, q0:q0 + HC], in_=stage_t[cb])

        # pass 1: width attention (sequences along w, one per h) -> acc1 = xs + ow
        axial_pass(xbf, w_sb["wq_w"], w_sb["wk_w"], w_sb["wv_w"], w_sb["wo_w"],
                   xf, acc1, transposed=False)
        # pass 2: height attention (sequences along h, one per w) -> out = acc1 + oh
        axial_pass(xtbf, w_sb["wq_h"], w_sb["wk_h"], w_sb["wv_h"], w_sb["wo_h"],
                   acc1, out, transposed=True)

    nc.compile()
    return nc


def _get_program():
    global _PROG
    if _PROG is None:
        _PROG = _build_program()
    return _PROG


def kernel(xs, Wq_h, Wk_h, Wv_h, Wo_h, Wq_w, Wk_w, Wv_w, Wo_w):
    from concourse.bass_utils import run_bass_kernel_spmd

    nc = _get_program()

    # pre-transpose weights to (c_in, c_out) and cast to bf16 (host-side prep)
    wmap = {
        "wq_w": Wq_w, "wk_w": Wk_w, "wv_w": Wv_w, "wo_w": Wo_w,
        "wq_h": Wq_h, "wk_h": Wk_h, "wv_h": Wv_h, "wo_h": Wo_h,
    }
    wt_np = {n: np.ascontiguousarray(w.T).astype(_BF16) for n, w in wmap.items()}

    xs = np.asarray(xs, dtype=np.float32)
    in_maps = []
    for b in range(NCORES):
        xb = np.ascontiguousarray(xs[b])                        # (C, H, W) f32
        xbf = xb.astype(_BF16)                                  # (C, H, W) bf16
        xtbf = np.ascontiguousarray(np.swapaxes(xb, 1, 2)).astype(_BF16)  # (C, W, H)
        in_maps.append({"xf": xb, "xbf": xbf, "xtbf": xtbf, **wt_np})

    res = run_bass_kernel_spmd(nc, in_maps, core_ids=list(range(NCORES)))
    return np.stack([res.results[b]["out"] for b in range(NCORES)], axis=0)
